# revision 6
# baseline (speedup 1.0000x reference)
"""Trainium2 Bass kernel for nn_ActorSpine (population-coding encoder MLP actor).

Reference computation (per sample):
  spine = sigmoid((state[:, :, None] - mean_enc) / std_enc)  # [B, 128, 10]
  a1 = relu(spine.reshape(B, 1280) @ W1.T + b1)              # [B, 2048]
  a2 = relu(a1 @ W2.T + b2)                                  # [B, 2048]
  a3 = a2 @ W3.T + b3                                        # [B, 320]
  raw = einsum('bak,ak->ba', a3.reshape(B, 32, 10), Wd[:, 0]) + bd
  out = tanh(raw)                                            # [B, 32]

Strategy: pure data parallel over 8 cores (2048 samples each).
Host-side folding:
  - decoder conv folds into W3: W3p[a, h] = sum_k Wd[a,0,k] * W3[a*10+k, h],
    b3p[a] = sum_k Wd[a,0,k]*b3[a*10+k] + bd[a]  -> final layer is [32, 2048]
  - encoder contraction index permuted j' = k*128 + d so spine k-tiles are
    plain per-partition sigmoid activations of stateT; W1 columns permuted to
    match.
Device: activations kept transposed [feature, batch]; fp16 matmul inputs
(full TensorE rate), fp32 PSUM accumulation, ScalarE fused bias+relu/tanh.
"""

import numpy as np

import concourse.bass as bass
import concourse.mybir as mybir
import concourse.tile as tile
from concourse import bacc
from concourse.bass_utils import run_bass_kernel_spmd

# Problem dims (hardcoded per harness contract)
B = 16384
D = 128
ENC_K = 10
ACT_DIM = 32
DEC_K = 10
H0 = 2048
H1 = 2048
NCORES = 8
BL = B // NCORES  # 2048 samples per core
NT = 512          # moving-dim tile (one PSUM bank of fp32)
NSUB = BL // NT   # 4
M1 = H0 // 128    # 16 m-tiles for layer 1
K1 = ENC_K        # 10 k-tiles for layer 1 (permuted encoder)
M2 = H1 // 128    # 16
K2 = H0 // 128    # 16
K3 = H1 // 128    # 16

F16 = mybir.dt.float16
F32 = mybir.dt.float32

_cached = {}


def _build_program():
    if "nc" in _cached:
        return _cached["nc"]

    nc = bacc.Bacc("TRN2", target_bir_lowering=False, debug=False,
                   num_devices=NCORES)

    stateT = nc.dram_tensor("stateT", [D, BL], F32, kind="ExternalInput")
    w1t = nc.dram_tensor("w1t", [M1, 128, K1, 128], F16, kind="ExternalInput")
    w2t = nc.dram_tensor("w2t", [M2, 128, K2, 128], F16, kind="ExternalInput")
    w3t = nc.dram_tensor("w3t", [128, K3, ACT_DIM], F16, kind="ExternalInput")
    # scalars layout (per partition p): [0:10] enc_scale, [10:20] enc_bias,
    # [20:36] b1, [36:52] b2, [52] b3p (partitions 0..31)
    scal = nc.dram_tensor("scal", [128, 53], F32, kind="ExternalInput")
    out = nc.dram_tensor("out", [ACT_DIM, BL], F32, kind="ExternalOutput")

    ADD = mybir.AluOpType.add
    MAX = mybir.AluOpType.max

    with tile.TileContext(nc) as tc:
        with (
            tc.tile_pool(name="consts", bufs=1) as consts,
            tc.tile_pool(name="acts", bufs=1) as acts,
            tc.tile_pool(name="h2p", bufs=4) as h2p,
            tc.tile_pool(name="w1p", bufs=2) as w1p,
            tc.tile_pool(name="w2p", bufs=2) as w2p,
            tc.tile_pool(name="outp", bufs=2) as outp,
            tc.tile_pool(name="psum", bufs=4, space="PSUM") as psum_pool,
            tc.tile_pool(name="psum3", bufs=1, space="PSUM") as psum3_pool,
        ):
            sc = consts.tile([128, 53], F32)
            nc.sync.dma_start(out=sc, in_=scal[:, :])

            # state DMA in column chunks so spine production starts early
            st = acts.tile([D, BL], F32, tag="state")
            for n in range(NSUB):
                nc.sync.dma_start(out=st[:, n * NT:(n + 1) * NT],
                                  in_=stateT[:, n * NT:(n + 1) * NT])

            psum3 = [psum3_pool.tile([ACT_DIM, NT], F32, tag=f"p3_{n}",
                                     name=f"p3_{n}")
                     for n in range(NSUB)]

            # ---- PE warmup: dummy matmuls on a zeroed tile so the HAM
            # clock-gate opens during the initial state/weight DMA window.
            wz = consts.tile([128, NT], F16, tag="warmzero")
            nc.vector.memset(wz, 0.0)
            for w in range(24):
                nc.tensor.matmul(
                    psum3[w % NSUB], wz[:, :ACT_DIM], wz,
                    start=(w < NSUB), stop=(w >= 24 - NSUB),
                    skip_group_check=True)

            w3sb = consts.tile([128, K3, ACT_DIM], F16, tag="w3")
            nc.sync.dma_start(out=w3sb, in_=w3t[:, :, :])

            # ---- encoder: spine[k][:, nslice] = sigmoid(state*scale_k + b_k)
            spine = [acts.tile([128, BL], F16, tag=f"spine{k}",
                               name=f"spine{k}")
                     for k in range(K1)]
            for n in range(NSUB):
                for k in range(K1):
                    nc.scalar.activation(
                        spine[k][:, n * NT:(n + 1) * NT],
                        st[:, n * NT:(n + 1) * NT],
                        mybir.ActivationFunctionType.Sigmoid,
                        bias=sc[:, 10 + k:11 + k], scale=sc[:, k:k + 1])

            # ---- layer 1: h1[m] = relu(W1p[m-block] @ spine + b1) on DVE
            h1 = []
            for m in range(M1):
                w1sb = w1p.tile([128, K1 * 128], F16, tag="w1")
                nc.sync.dma_start(
                    out=w1sb, in_=w1t[m].rearrange("p k j -> p (k j)"))
                h1m = acts.tile([128, BL], F16, tag=f"h1_{m}")
                for n in range(NSUB):
                    ps = psum_pool.tile([128, NT], F32, tag="ps")
                    for k in range(K1):
                        nc.tensor.matmul(
                            ps, w1sb[:, k * 128:(k + 1) * 128],
                            spine[k][:, n * NT:(n + 1) * NT],
                            start=(k == 0), stop=(k == K1 - 1))
                    nc.vector.tensor_scalar(
                        h1m[:, n * NT:(n + 1) * NT], ps,
                        sc[:, 20 + m:21 + m], 0.0, ADD, MAX)
                h1.append(h1m)

            # ---- layer 2 (+ layer 3 lagged two groups behind the relu)
            pending = []

            def emit_l3(args):
                m, n, h2m = args
                nc.tensor.matmul(
                    psum3[n], w3sb[:, m, :], h2m[:, n * NT:(n + 1) * NT],
                    start=(m == 0), stop=(m == M2 - 1),
                    skip_group_check=True)

            for m in range(M2):
                w2sb = w2p.tile([128, K2 * 128], F16, tag="w2")
                nc.sync.dma_start(
                    out=w2sb, in_=w2t[m].rearrange("p k j -> p (k j)"))
                h2m = h2p.tile([128, BL], F16, tag="h2")
                for n in range(NSUB):
                    ps = psum_pool.tile([128, NT], F32, tag="ps")
                    for k in range(K2):
                        nc.tensor.matmul(
                            ps, w2sb[:, k * 128:(k + 1) * 128],
                            h1[k][:, n * NT:(n + 1) * NT],
                            start=(k == 0), stop=(k == K2 - 1))
                    nc.vector.tensor_scalar(
                        h2m[:, n * NT:(n + 1) * NT], ps,
                        sc[:, 36 + m:37 + m], 0.0, ADD, MAX)
                    pending.append((m, n, h2m))
                    if len(pending) > 2:
                        emit_l3(pending.pop(0))
            while pending:
                emit_l3(pending.pop(0))

            # ---- output: tanh(raw + b3p)
            for n in range(NSUB):
                ot = outp.tile([ACT_DIM, NT], F32, tag="ot")
                nc.scalar.activation(
                    ot, psum3[n], mybir.ActivationFunctionType.Tanh,
                    bias=sc[:ACT_DIM, 52:53])
                nc.sync.dma_start(out=out[:, n * NT:(n + 1) * NT], in_=ot)

    nc.compile()
    _cached["nc"] = nc
    return nc


def _prep_inputs(state, mean_enc, std_enc, W1, b1, W2, b2, W3, b3, Wd, bd):
    f32 = np.float32
    state = np.asarray(state, f32)
    mean_enc = np.asarray(mean_enc, f32)
    std_enc = np.asarray(std_enc, f32)
    W1 = np.asarray(W1, f32)
    b1 = np.asarray(b1, f32)
    W2 = np.asarray(W2, f32)
    b2 = np.asarray(b2, f32)
    W3 = np.asarray(W3, f32)
    b3 = np.asarray(b3, f32)
    Wd = np.asarray(Wd, f32)
    bd = np.asarray(bd, f32)

    # Fold decoder grouped conv into layer 3
    wd = Wd[:, 0, :]                                   # [32, 10]
    W3p = np.einsum("ak,akh->ah", wd, W3.reshape(ACT_DIM, DEC_K, H1))
    b3p = (b3.reshape(ACT_DIM, DEC_K) * wd).sum(1) + bd  # [32]

    # Permute encoder contraction: j' = k*128 + d
    W1p = W1.reshape(H0, D, ENC_K).transpose(0, 2, 1).reshape(H0, D * ENC_K)

    # Pre-tiled weight layouts: [m, p, k, j] = lhsT tile stack
    w1t = np.ascontiguousarray(
        W1p.reshape(M1, 128, K1, 128).transpose(0, 3, 2, 1).astype(np.float16))
    w2t = np.ascontiguousarray(
        W2.reshape(M2, 128, K2, 128).transpose(0, 3, 2, 1).astype(np.float16))
    w3t = np.ascontiguousarray(
        W3p.reshape(ACT_DIM, K3, 128).transpose(2, 1, 0).astype(np.float16))

    scal = np.zeros((128, 53), f32)
    scal[:, 0:10] = 1.0 / std_enc[0]                   # enc scale [128, 10]
    scal[:, 10:20] = -mean_enc[0] / std_enc[0]         # enc bias
    scal[:, 20:36] = b1.reshape(M1, 128).T
    scal[:, 36:52] = b2.reshape(M2, 128).T
    scal[:ACT_DIM, 52] = b3p

    in_maps = []
    for c in range(NCORES):
        shard = np.ascontiguousarray(state[c * BL:(c + 1) * BL].T)  # [128, BL]
        in_maps.append({
            "stateT": shard, "w1t": w1t, "w2t": w2t, "w3t": w3t, "scal": scal,
        })
    return in_maps


def kernel(**inputs):
    nc = _build_program()
    in_maps = _prep_inputs(**inputs)
    res = run_bass_kernel_spmd(nc, in_maps, core_ids=list(range(NCORES)))
    out = np.concatenate(
        [res.results[c]["out"].T for c in range(NCORES)], axis=0)
    return np.ascontiguousarray(out.astype(np.float32))


if __name__ == "__main__":
    rng = np.random.default_rng(0)
    state = rng.standard_normal((B, D), dtype=np.float32)
    mean = np.broadcast_to(
        np.linspace(-3, 3, ENC_K, dtype=np.float32), (1, D, ENC_K)).copy()
    std = np.full((1, D, ENC_K), 0.3872983346207417, np.float32)

    def lin(fan_in, fan_out):
        bound = 1 / np.sqrt(fan_in)
        return (rng.uniform(-bound, bound, (fan_out, fan_in)).astype(np.float32),
                rng.uniform(-bound, bound, fan_out).astype(np.float32))

    W1, b1 = lin(D * ENC_K, H0)
    W2, b2 = lin(H0, H1)
    W3, b3 = lin(H1, ACT_DIM * DEC_K)
    Wd = rng.uniform(-0.3, 0.3, (ACT_DIM, 1, DEC_K)).astype(np.float32)
    bd = rng.uniform(-0.3, 0.3, ACT_DIM).astype(np.float32)

    outp = kernel(state=state, mean_enc=mean, std_enc=std, W1=W1, b1=b1,
                  W2=W2, b2=b2, W3=W3, b3=b3, Wd=Wd, bd=bd)

    # numpy reference
    spine = 1 / (1 + np.exp(-(state[:, :, None] - mean) / std))
    a = np.maximum(spine.reshape(B, -1) @ W1.T + b1, 0)
    a = np.maximum(a @ W2.T + b2, 0)
    a = a @ W3.T + b3
    raw = np.einsum("bak,ak->ba", a.reshape(B, ACT_DIM, DEC_K), Wd[:, 0]) + bd
    ref = np.tanh(raw)
    rel = np.linalg.norm(outp - ref) / np.linalg.norm(ref)
    print("rel err:", rel, "max abs diff:", np.abs(outp - ref).max())


# revision 9
# speedup vs baseline: 1.0313x; 1.0313x over previous
"""Trainium2 Bass kernel for nn_ActorSpine (population-coding encoder MLP actor).

Reference computation (per sample):
  spine = sigmoid((state[:, :, None] - mean_enc) / std_enc)  # [B, 128, 10]
  a1 = relu(spine.reshape(B, 1280) @ W1.T + b1)              # [B, 2048]
  a2 = relu(a1 @ W2.T + b2)                                  # [B, 2048]
  a3 = a2 @ W3.T + b3                                        # [B, 320]
  raw = einsum('bak,ak->ba', a3.reshape(B, 32, 10), Wd[:, 0]) + bd
  out = tanh(raw)                                            # [B, 32]

Strategy: pure data parallel over 8 cores (2048 samples each).
Host-side folding:
  - decoder conv folds into W3: W3p[a, h] = sum_k Wd[a,0,k] * W3[a*10+k, h],
    b3p[a] = sum_k Wd[a,0,k]*b3[a*10+k] + bd[a]  -> final layer is [32, 2048]
  - encoder contraction index permuted j' = k*128 + d so spine k-tiles are
    plain per-partition sigmoid activations of stateT; W1 columns permuted to
    match.
Device: activations kept transposed [feature, batch]; fp16 matmul inputs
(full TensorE rate), fp32 PSUM accumulation, ScalarE fused bias+relu/tanh.
"""

import numpy as np

import concourse.bass as bass
import concourse.mybir as mybir
import concourse.tile as tile
from concourse import bacc
from concourse.bass_utils import run_bass_kernel_spmd

# Problem dims (hardcoded per harness contract)
B = 16384
D = 128
ENC_K = 10
ACT_DIM = 32
DEC_K = 10
H0 = 2048
H1 = 2048
NCORES = 8
BL = B // NCORES  # 2048 samples per core
NT = 512          # moving-dim tile (one PSUM bank of fp32)
NSUB = BL // NT   # 4
M1 = H0 // 128    # 16 m-tiles for layer 1
K1 = ENC_K        # 10 k-tiles for layer 1 (permuted encoder)
M2 = H1 // 128    # 16
K2 = H0 // 128    # 16
K3 = H1 // 128    # 16

F16 = mybir.dt.float16
F32 = mybir.dt.float32

_cached = {}


def _build_program():
    if "nc" in _cached:
        return _cached["nc"]

    nc = bacc.Bacc("TRN2", target_bir_lowering=False, debug=False,
                   num_devices=NCORES)

    stateT = nc.dram_tensor("stateT", [D, BL], F32, kind="ExternalInput")
    w1t = nc.dram_tensor("w1t", [M1, 128, K1, 128], F16, kind="ExternalInput")
    w2t = nc.dram_tensor("w2t", [M2, 128, K2, 128], F16, kind="ExternalInput")
    w3t = nc.dram_tensor("w3t", [128, K3, ACT_DIM], F16, kind="ExternalInput")
    # scalars layout (per partition p): [0:10] enc_scale, [10:20] enc_bias,
    # [20:36] b1, [36:52] b2, [52] b3p (partitions 0..31)
    scal = nc.dram_tensor("scal", [128, 53], F32, kind="ExternalInput")
    out = nc.dram_tensor("out", [ACT_DIM, BL], F32, kind="ExternalOutput")

    ADD = mybir.AluOpType.add
    MAX = mybir.AluOpType.max

    with tile.TileContext(nc) as tc:
        with (
            tc.tile_pool(name="consts", bufs=1) as consts,
            tc.tile_pool(name="acts", bufs=1) as acts,
            tc.tile_pool(name="h2p", bufs=4) as h2p,
            tc.tile_pool(name="w1p", bufs=1) as w1p,
            tc.tile_pool(name="w2p", bufs=3) as w2p,
            tc.tile_pool(name="outp", bufs=2) as outp,
            tc.tile_pool(name="psum", bufs=4, space="PSUM") as psum_pool,
            tc.tile_pool(name="psum3", bufs=1, space="PSUM") as psum3_pool,
        ):
            sc = consts.tile([128, 53], F32)
            nc.sync.dma_start(out=sc, in_=scal[:, :])

            # state DMA in column chunks so spine production starts early
            st = acts.tile([D, BL], F32, tag="state")
            for n in range(NSUB):
                nc.sync.dma_start(out=st[:, n * NT:(n + 1) * NT],
                                  in_=stateT[:, n * NT:(n + 1) * NT])

            psum3 = [psum3_pool.tile([ACT_DIM, NT], F32, tag=f"p3_{n}",
                                     name=f"p3_{n}")
                     for n in range(NSUB)]

            # ---- PE warmup: dummy matmuls on a zeroed tile so the HAM
            # clock-gate opens during the initial state/weight DMA window.
            wz = consts.tile([128, NT], F16, tag="warmzero")
            nc.vector.memset(wz, 0.0)
            NWARM = 40
            for w in range(NWARM):
                nc.tensor.matmul(
                    psum3[w % NSUB], wz[:, :ACT_DIM], wz,
                    start=(w < NSUB), stop=(w >= NWARM - NSUB),
                    skip_group_check=True)

            w3sb = consts.tile([128, K3, ACT_DIM], F16, tag="w3")
            nc.sync.dma_start(out=w3sb, in_=w3t[:, :, :])

            # ---- encoder: spine[k][:, nslice] = sigmoid(state*scale_k + b_k)
            spine = [acts.tile([128, BL], F16, tag=f"spine{k}",
                               name=f"spine{k}")
                     for k in range(K1)]
            for n in range(NSUB):
                for k in range(K1):
                    nc.scalar.activation(
                        spine[k][:, n * NT:(n + 1) * NT],
                        st[:, n * NT:(n + 1) * NT],
                        mybir.ActivationFunctionType.Sigmoid,
                        bias=sc[:, 10 + k:11 + k], scale=sc[:, k:k + 1])

            # ---- layer 1: h1[m] = relu(W1p[m-block] @ spine + b1) on DVE.
            # W1 resident in SBUF; n-outer so one spine column-chunk unlocks
            # a full sweep of 160 matmuls (avoids ScalarE-ramp starvation).
            w1sb = []
            for m in range(M1):
                w1m = w1p.tile([128, K1 * 128], F16, tag=f"w1_{m}",
                               name=f"w1_{m}")
                nc.sync.dma_start(
                    out=w1m, in_=w1t[m].rearrange("p k j -> p (k j)"))
                w1sb.append(w1m)
            h1 = [acts.tile([128, BL], F16, tag=f"h1_{m}", name=f"h1_{m}")
                  for m in range(M1)]
            for n in range(NSUB):
                for m in range(M1):
                    ps = psum_pool.tile([128, NT], F32, tag="ps")
                    for k in range(K1):
                        nc.tensor.matmul(
                            ps, w1sb[m][:, k * 128:(k + 1) * 128],
                            spine[k][:, n * NT:(n + 1) * NT],
                            start=(k == 0), stop=(k == K1 - 1))
                    nc.vector.tensor_scalar(
                        h1[m][:, n * NT:(n + 1) * NT], ps,
                        sc[:, 20 + m:21 + m], 0.0, ADD, MAX)

            # ---- layer 2 (+ layer 3 lagged two groups behind the relu)
            pending = []

            def emit_l3(args):
                m, n, h2m = args
                nc.tensor.matmul(
                    psum3[n], w3sb[:, m, :], h2m[:, n * NT:(n + 1) * NT],
                    start=(m == 0), stop=(m == M2 - 1),
                    skip_group_check=True)

            for m in range(M2):
                w2sb = w2p.tile([128, K2 * 128], F16, tag="w2")
                nc.sync.dma_start(
                    out=w2sb, in_=w2t[m].rearrange("p k j -> p (k j)"))
                h2m = h2p.tile([128, BL], F16, tag="h2")
                for n in range(NSUB):
                    ps = psum_pool.tile([128, NT], F32, tag="ps")
                    for k in range(K2):
                        nc.tensor.matmul(
                            ps, w2sb[:, k * 128:(k + 1) * 128],
                            h1[k][:, n * NT:(n + 1) * NT],
                            start=(k == 0), stop=(k == K2 - 1))
                    nc.vector.tensor_scalar(
                        h2m[:, n * NT:(n + 1) * NT], ps,
                        sc[:, 36 + m:37 + m], 0.0, ADD, MAX)
                    pending.append((m, n, h2m))
                    if len(pending) > 2:
                        emit_l3(pending.pop(0))
            while pending:
                emit_l3(pending.pop(0))

            # ---- output: tanh(raw + b3p)
            for n in range(NSUB):
                ot = outp.tile([ACT_DIM, NT], F32, tag="ot")
                nc.scalar.activation(
                    ot, psum3[n], mybir.ActivationFunctionType.Tanh,
                    bias=sc[:ACT_DIM, 52:53])
                nc.sync.dma_start(out=out[:, n * NT:(n + 1) * NT], in_=ot)

    nc.compile()
    _cached["nc"] = nc
    return nc


def _prep_inputs(state, mean_enc, std_enc, W1, b1, W2, b2, W3, b3, Wd, bd):
    f32 = np.float32
    state = np.asarray(state, f32)
    mean_enc = np.asarray(mean_enc, f32)
    std_enc = np.asarray(std_enc, f32)
    W1 = np.asarray(W1, f32)
    b1 = np.asarray(b1, f32)
    W2 = np.asarray(W2, f32)
    b2 = np.asarray(b2, f32)
    W3 = np.asarray(W3, f32)
    b3 = np.asarray(b3, f32)
    Wd = np.asarray(Wd, f32)
    bd = np.asarray(bd, f32)

    # Fold decoder grouped conv into layer 3
    wd = Wd[:, 0, :]                                   # [32, 10]
    W3p = np.einsum("ak,akh->ah", wd, W3.reshape(ACT_DIM, DEC_K, H1))
    b3p = (b3.reshape(ACT_DIM, DEC_K) * wd).sum(1) + bd  # [32]

    # Permute encoder contraction: j' = k*128 + d
    W1p = W1.reshape(H0, D, ENC_K).transpose(0, 2, 1).reshape(H0, D * ENC_K)

    # Pre-tiled weight layouts: [m, p, k, j] = lhsT tile stack
    w1t = np.ascontiguousarray(
        W1p.reshape(M1, 128, K1, 128).transpose(0, 3, 2, 1).astype(np.float16))
    w2t = np.ascontiguousarray(
        W2.reshape(M2, 128, K2, 128).transpose(0, 3, 2, 1).astype(np.float16))
    w3t = np.ascontiguousarray(
        W3p.reshape(ACT_DIM, K3, 128).transpose(2, 1, 0).astype(np.float16))

    scal = np.zeros((128, 53), f32)
    scal[:, 0:10] = 1.0 / std_enc[0]                   # enc scale [128, 10]
    scal[:, 10:20] = -mean_enc[0] / std_enc[0]         # enc bias
    scal[:, 20:36] = b1.reshape(M1, 128).T
    scal[:, 36:52] = b2.reshape(M2, 128).T
    scal[:ACT_DIM, 52] = b3p

    in_maps = []
    for c in range(NCORES):
        shard = np.ascontiguousarray(state[c * BL:(c + 1) * BL].T)  # [128, BL]
        in_maps.append({
            "stateT": shard, "w1t": w1t, "w2t": w2t, "w3t": w3t, "scal": scal,
        })
    return in_maps


def kernel(**inputs):
    nc = _build_program()
    in_maps = _prep_inputs(**inputs)
    res = run_bass_kernel_spmd(nc, in_maps, core_ids=list(range(NCORES)))
    out = np.concatenate(
        [res.results[c]["out"].T for c in range(NCORES)], axis=0)
    return np.ascontiguousarray(out.astype(np.float32))


if __name__ == "__main__":
    rng = np.random.default_rng(0)
    state = rng.standard_normal((B, D), dtype=np.float32)
    mean = np.broadcast_to(
        np.linspace(-3, 3, ENC_K, dtype=np.float32), (1, D, ENC_K)).copy()
    std = np.full((1, D, ENC_K), 0.3872983346207417, np.float32)

    def lin(fan_in, fan_out):
        bound = 1 / np.sqrt(fan_in)
        return (rng.uniform(-bound, bound, (fan_out, fan_in)).astype(np.float32),
                rng.uniform(-bound, bound, fan_out).astype(np.float32))

    W1, b1 = lin(D * ENC_K, H0)
    W2, b2 = lin(H0, H1)
    W3, b3 = lin(H1, ACT_DIM * DEC_K)
    Wd = rng.uniform(-0.3, 0.3, (ACT_DIM, 1, DEC_K)).astype(np.float32)
    bd = rng.uniform(-0.3, 0.3, ACT_DIM).astype(np.float32)

    outp = kernel(state=state, mean_enc=mean, std_enc=std, W1=W1, b1=b1,
                  W2=W2, b2=b2, W3=W3, b3=b3, Wd=Wd, bd=bd)

    # numpy reference
    spine = 1 / (1 + np.exp(-(state[:, :, None] - mean) / std))
    a = np.maximum(spine.reshape(B, -1) @ W1.T + b1, 0)
    a = np.maximum(a @ W2.T + b2, 0)
    a = a @ W3.T + b3
    raw = np.einsum("bak,ak->ba", a.reshape(B, ACT_DIM, DEC_K), Wd[:, 0]) + bd
    ref = np.tanh(raw)
    rel = np.linalg.norm(outp - ref) / np.linalg.norm(ref)
    print("rel err:", rel, "max abs diff:", np.abs(outp - ref).max())


# revision 10
# speedup vs baseline: 1.0355x; 1.0041x over previous
"""Trainium2 Bass kernel for nn_ActorSpine (population-coding encoder MLP actor).

Reference computation (per sample):
  spine = sigmoid((state[:, :, None] - mean_enc) / std_enc)  # [B, 128, 10]
  a1 = relu(spine.reshape(B, 1280) @ W1.T + b1)              # [B, 2048]
  a2 = relu(a1 @ W2.T + b2)                                  # [B, 2048]
  a3 = a2 @ W3.T + b3                                        # [B, 320]
  raw = einsum('bak,ak->ba', a3.reshape(B, 32, 10), Wd[:, 0]) + bd
  out = tanh(raw)                                            # [B, 32]

Strategy: pure data parallel over 8 cores (2048 samples each).
Host-side folding:
  - decoder conv folds into W3: W3p[a, h] = sum_k Wd[a,0,k] * W3[a*10+k, h],
    b3p[a] = sum_k Wd[a,0,k]*b3[a*10+k] + bd[a]  -> final layer is [32, 2048]
  - encoder contraction index permuted j' = k*128 + d so spine k-tiles are
    plain per-partition sigmoid activations of stateT; W1 columns permuted to
    match.
Device: activations kept transposed [feature, batch]; fp16 matmul inputs
(full TensorE rate), fp32 PSUM accumulation, ScalarE fused bias+relu/tanh.
"""

import numpy as np

import concourse.bass as bass
import concourse.mybir as mybir
import concourse.tile as tile
from concourse import bacc
from concourse.bass_utils import run_bass_kernel_spmd

# Problem dims (hardcoded per harness contract)
B = 16384
D = 128
ENC_K = 10
ACT_DIM = 32
DEC_K = 10
H0 = 2048
H1 = 2048
NCORES = 8
BL = B // NCORES  # 2048 samples per core
NT = 512          # moving-dim tile (one PSUM bank of fp32)
NSUB = BL // NT   # 4
M1 = H0 // 128    # 16 m-tiles for layer 1
K1 = ENC_K        # 10 k-tiles for layer 1 (permuted encoder)
M2 = H1 // 128    # 16
K2 = H0 // 128    # 16
K3 = H1 // 128    # 16

F16 = mybir.dt.float16
F32 = mybir.dt.float32

_cached = {}


def _build_program():
    if "nc" in _cached:
        return _cached["nc"]

    nc = bacc.Bacc("TRN2", target_bir_lowering=False, debug=False,
                   num_devices=NCORES)

    stateT = nc.dram_tensor("stateT", [D, BL], F32, kind="ExternalInput")
    w1t = nc.dram_tensor("w1t", [M1, 128, K1, 128], F16, kind="ExternalInput")
    w2t = nc.dram_tensor("w2t", [M2, 128, K2, 128], F16, kind="ExternalInput")
    w3t = nc.dram_tensor("w3t", [128, K3, ACT_DIM], F16, kind="ExternalInput")
    # scalars layout (per partition p): [0:10] enc_scale, [10:20] enc_bias,
    # [20:36] b1, [36:52] b2, [52] b3p (partitions 0..31)
    scal = nc.dram_tensor("scal", [128, 53], F32, kind="ExternalInput")
    out = nc.dram_tensor("out", [ACT_DIM, BL], F32, kind="ExternalOutput")

    ADD = mybir.AluOpType.add
    MAX = mybir.AluOpType.max

    with tile.TileContext(nc) as tc:
        with (
            tc.tile_pool(name="consts", bufs=1) as consts,
            tc.tile_pool(name="acts", bufs=1) as acts,
            tc.tile_pool(name="h2p", bufs=4) as h2p,
            tc.tile_pool(name="w1p", bufs=1) as w1p,
            tc.tile_pool(name="w2p", bufs=3) as w2p,
            tc.tile_pool(name="outp", bufs=2) as outp,
            tc.tile_pool(name="psum", bufs=6, space="PSUM") as psum_pool,
            tc.tile_pool(name="psum3", bufs=1, space="PSUM") as psum3_pool,
        ):
            sc = consts.tile([128, 53], F32)
            nc.sync.dma_start(out=sc, in_=scal[:, :])

            # state DMA in column chunks so spine production starts early
            st = acts.tile([D, BL], F32, tag="state")
            for n in range(NSUB):
                nc.sync.dma_start(out=st[:, n * NT:(n + 1) * NT],
                                  in_=stateT[:, n * NT:(n + 1) * NT])

            # ---- PE warmup: dummy matmuls on a zeroed tile so the HAM
            # clock-gate opens during the initial state/weight DMA window.
            wz = consts.tile([128, NT], F16, tag="warmzero")
            nc.vector.memset(wz, 0.0)
            NWARM = 16
            wps = psum_pool.tile([128, NT], F32, tag="ps", name="warm_ps")
            for w in range(NWARM):
                nc.tensor.matmul(
                    wps, wz[:, :128], wz,
                    start=(w == 0), stop=(w == NWARM - 1),
                    skip_group_check=True)

            w3sb = consts.tile([128, K3, ACT_DIM], F16, tag="w3")
            nc.sync.dma_start(out=w3sb, in_=w3t[:, :, :])

            # W1 resident in SBUF (one block per m-tile, loaded once)
            w1sb = []
            for m in range(M1):
                w1m = w1p.tile([128, K1 * 128], F16, tag=f"w1_{m}",
                               name=f"w1_{m}")
                nc.sync.dma_start(
                    out=w1m, in_=w1t[m].rearrange("p k j -> p (k j)"))
                w1sb.append(w1m)

            spine = [acts.tile([128, BL], F16, tag=f"spine{k}",
                               name=f"spine{k}")
                     for k in range(K1)]
            h1 = [acts.tile([128, BL], F16, tag=f"h1_{m}", name=f"h1_{m}")
                  for m in range(M1)]

            # layer-3 matmuls lag two (m, n) groups behind their relu so the
            # PE never waits on a freshly written h2 tile.
            pending = []

            def emit_l3(args):
                m, n, h2m, p3 = args
                nc.tensor.matmul(
                    p3, w3sb[:, m, :], h2m,
                    start=(m == 0), stop=(m == M2 - 1),
                    skip_group_check=True)
                if m == M2 - 1:
                    ot = outp.tile([ACT_DIM, NT], F32, tag="ot",
                                   name=f"ot_{n}")
                    nc.scalar.activation(
                        ot, p3, mybir.ActivationFunctionType.Tanh,
                        bias=sc[:ACT_DIM, 52:53])
                    nc.sync.dma_start(out=out[:, n * NT:(n + 1) * NT],
                                      in_=ot)

            # ---- fully interleaved per-column-chunk sweeps:
            # sigmoid(n) -> L1 m-sweep(n) -> L2 m-sweep(n) [+ lagged L3/tanh]
            for n in range(NSUB):
                ns = slice(n * NT, (n + 1) * NT)
                for k in range(K1):
                    nc.scalar.activation(
                        spine[k][:, ns], st[:, ns],
                        mybir.ActivationFunctionType.Sigmoid,
                        bias=sc[:, 10 + k:11 + k], scale=sc[:, k:k + 1])

                for m in range(M1):
                    ps = psum_pool.tile([128, NT], F32, tag="ps")
                    for k in range(K1):
                        nc.tensor.matmul(
                            ps, w1sb[m][:, k * 128:(k + 1) * 128],
                            spine[k][:, ns],
                            start=(k == 0), stop=(k == K1 - 1))
                    nc.vector.tensor_scalar(
                        h1[m][:, ns], ps, sc[:, 20 + m:21 + m], 0.0, ADD, MAX)

                p3 = psum3_pool.tile([ACT_DIM, NT], F32, tag=f"p3_{n % 2}",
                                     name=f"p3_{n}")
                for m in range(M2):
                    w2sb = w2p.tile([128, K2 * 128], F16, tag="w2")
                    nc.sync.dma_start(
                        out=w2sb, in_=w2t[m].rearrange("p k j -> p (k j)"))
                    ps = psum_pool.tile([128, NT], F32, tag="ps")
                    for k in range(K2):
                        nc.tensor.matmul(
                            ps, w2sb[:, k * 128:(k + 1) * 128],
                            h1[k][:, ns],
                            start=(k == 0), stop=(k == K2 - 1))
                    h2m = h2p.tile([128, NT], F16, tag="h2")
                    nc.vector.tensor_scalar(
                        h2m, ps, sc[:, 36 + m:37 + m], 0.0, ADD, MAX)
                    pending.append((m, n, h2m, p3))
                    if len(pending) > 2:
                        emit_l3(pending.pop(0))
            while pending:
                emit_l3(pending.pop(0))

    nc.compile()
    _cached["nc"] = nc
    return nc


def _prep_inputs(state, mean_enc, std_enc, W1, b1, W2, b2, W3, b3, Wd, bd):
    f32 = np.float32
    state = np.asarray(state, f32)
    mean_enc = np.asarray(mean_enc, f32)
    std_enc = np.asarray(std_enc, f32)
    W1 = np.asarray(W1, f32)
    b1 = np.asarray(b1, f32)
    W2 = np.asarray(W2, f32)
    b2 = np.asarray(b2, f32)
    W3 = np.asarray(W3, f32)
    b3 = np.asarray(b3, f32)
    Wd = np.asarray(Wd, f32)
    bd = np.asarray(bd, f32)

    # Fold decoder grouped conv into layer 3
    wd = Wd[:, 0, :]                                   # [32, 10]
    W3p = np.einsum("ak,akh->ah", wd, W3.reshape(ACT_DIM, DEC_K, H1))
    b3p = (b3.reshape(ACT_DIM, DEC_K) * wd).sum(1) + bd  # [32]

    # Permute encoder contraction: j' = k*128 + d
    W1p = W1.reshape(H0, D, ENC_K).transpose(0, 2, 1).reshape(H0, D * ENC_K)

    # Pre-tiled weight layouts: [m, p, k, j] = lhsT tile stack
    w1t = np.ascontiguousarray(
        W1p.reshape(M1, 128, K1, 128).transpose(0, 3, 2, 1).astype(np.float16))
    w2t = np.ascontiguousarray(
        W2.reshape(M2, 128, K2, 128).transpose(0, 3, 2, 1).astype(np.float16))
    w3t = np.ascontiguousarray(
        W3p.reshape(ACT_DIM, K3, 128).transpose(2, 1, 0).astype(np.float16))

    scal = np.zeros((128, 53), f32)
    scal[:, 0:10] = 1.0 / std_enc[0]                   # enc scale [128, 10]
    scal[:, 10:20] = -mean_enc[0] / std_enc[0]         # enc bias
    scal[:, 20:36] = b1.reshape(M1, 128).T
    scal[:, 36:52] = b2.reshape(M2, 128).T
    scal[:ACT_DIM, 52] = b3p

    in_maps = []
    for c in range(NCORES):
        shard = np.ascontiguousarray(state[c * BL:(c + 1) * BL].T)  # [128, BL]
        in_maps.append({
            "stateT": shard, "w1t": w1t, "w2t": w2t, "w3t": w3t, "scal": scal,
        })
    return in_maps


def kernel(**inputs):
    nc = _build_program()
    in_maps = _prep_inputs(**inputs)
    res = run_bass_kernel_spmd(nc, in_maps, core_ids=list(range(NCORES)))
    out = np.concatenate(
        [res.results[c]["out"].T for c in range(NCORES)], axis=0)
    return np.ascontiguousarray(out.astype(np.float32))


if __name__ == "__main__":
    rng = np.random.default_rng(0)
    state = rng.standard_normal((B, D), dtype=np.float32)
    mean = np.broadcast_to(
        np.linspace(-3, 3, ENC_K, dtype=np.float32), (1, D, ENC_K)).copy()
    std = np.full((1, D, ENC_K), 0.3872983346207417, np.float32)

    def lin(fan_in, fan_out):
        bound = 1 / np.sqrt(fan_in)
        return (rng.uniform(-bound, bound, (fan_out, fan_in)).astype(np.float32),
                rng.uniform(-bound, bound, fan_out).astype(np.float32))

    W1, b1 = lin(D * ENC_K, H0)
    W2, b2 = lin(H0, H1)
    W3, b3 = lin(H1, ACT_DIM * DEC_K)
    Wd = rng.uniform(-0.3, 0.3, (ACT_DIM, 1, DEC_K)).astype(np.float32)
    bd = rng.uniform(-0.3, 0.3, ACT_DIM).astype(np.float32)

    outp = kernel(state=state, mean_enc=mean, std_enc=std, W1=W1, b1=b1,
                  W2=W2, b2=b2, W3=W3, b3=b3, Wd=Wd, bd=bd)

    # numpy reference
    spine = 1 / (1 + np.exp(-(state[:, :, None] - mean) / std))
    a = np.maximum(spine.reshape(B, -1) @ W1.T + b1, 0)
    a = np.maximum(a @ W2.T + b2, 0)
    a = a @ W3.T + b3
    raw = np.einsum("bak,ak->ba", a.reshape(B, ACT_DIM, DEC_K), Wd[:, 0]) + bd
    ref = np.tanh(raw)
    rel = np.linalg.norm(outp - ref) / np.linalg.norm(ref)
    print("rel err:", rel, "max abs diff:", np.abs(outp - ref).max())


# revision 13
# speedup vs baseline: 1.0366x; 1.0010x over previous
"""Trainium2 Bass kernel for nn_ActorSpine (population-coding encoder MLP actor).

Reference computation (per sample):
  spine = sigmoid((state[:, :, None] - mean_enc) / std_enc)  # [B, 128, 10]
  a1 = relu(spine.reshape(B, 1280) @ W1.T + b1)              # [B, 2048]
  a2 = relu(a1 @ W2.T + b2)                                  # [B, 2048]
  a3 = a2 @ W3.T + b3                                        # [B, 320]
  raw = einsum('bak,ak->ba', a3.reshape(B, 32, 10), Wd[:, 0]) + bd
  out = tanh(raw)                                            # [B, 32]

Strategy: pure data parallel over 8 cores (2048 samples each).
Host-side folding:
  - decoder conv folds into W3: W3p[a, h] = sum_k Wd[a,0,k] * W3[a*10+k, h],
    b3p[a] = sum_k Wd[a,0,k]*b3[a*10+k] + bd[a]  -> final layer is [32, 2048]
  - encoder contraction index permuted j' = k*128 + d so spine k-tiles are
    plain per-partition sigmoid activations of stateT; W1 columns permuted to
    match.
Device: activations kept transposed [feature, batch]; fp16 matmul inputs
(full TensorE rate), fp32 PSUM accumulation, ScalarE fused bias+relu/tanh.
"""

import numpy as np

import concourse.bass as bass
import concourse.mybir as mybir
import concourse.tile as tile
from concourse import bacc
from concourse.bass_utils import run_bass_kernel_spmd

# Problem dims (hardcoded per harness contract)
B = 16384
D = 128
ENC_K = 10
ACT_DIM = 32
DEC_K = 10
H0 = 2048
H1 = 2048
NCORES = 8
BL = B // NCORES  # 2048 samples per core
NT = 512          # moving-dim tile (one PSUM bank of fp32)
NSUB = BL // NT   # 4
M1 = H0 // 128    # 16 m-tiles for layer 1
K1 = ENC_K        # 10 k-tiles for layer 1 (permuted encoder)
M2 = H1 // 128    # 16
K2 = H0 // 128    # 16
K3 = H1 // 128    # 16

F16 = mybir.dt.float16
F32 = mybir.dt.float32

_cached = {}


def _build_program():
    if "nc" in _cached:
        return _cached["nc"]

    nc = bacc.Bacc("TRN2", target_bir_lowering=False, debug=False,
                   num_devices=NCORES)

    stateT = nc.dram_tensor("stateT", [D, BL], F32, kind="ExternalInput")
    w1t = nc.dram_tensor("w1t", [M1, 128, K1, 128], F16, kind="ExternalInput")
    w2t = nc.dram_tensor("w2t", [M2, 128, K2, 128], F16, kind="ExternalInput")
    w3t = nc.dram_tensor("w3t", [128, K3, ACT_DIM], F16, kind="ExternalInput")
    # scalars layout (per partition p): [0:10] enc_scale, [10:20] enc_bias,
    # [20:36] b1, [36:52] b2, [52] b3p (partitions 0..31)
    scal = nc.dram_tensor("scal", [128, 53], F32, kind="ExternalInput")
    out = nc.dram_tensor("out", [ACT_DIM, BL], F32, kind="ExternalOutput")

    ADD = mybir.AluOpType.add
    MAX = mybir.AluOpType.max

    with tile.TileContext(nc) as tc:
        with (
            tc.tile_pool(name="consts", bufs=1) as consts,
            tc.tile_pool(name="acts", bufs=1) as acts,
            tc.tile_pool(name="h2p", bufs=4) as h2p,
            tc.tile_pool(name="w1p", bufs=1) as w1p,
            tc.tile_pool(name="w2p", bufs=3) as w2p,
            tc.tile_pool(name="outp", bufs=2) as outp,
            tc.tile_pool(name="psum", bufs=1, space="PSUM") as psum_pool,
            tc.tile_pool(name="psum3", bufs=1, space="PSUM") as psum3_pool,
        ):
            sc = consts.tile([128, 53], F32)
            nc.sync.dma_start(out=sc, in_=scal[:, :])

            # state DMA in column chunks so spine production starts early
            st = acts.tile([D, BL], F32, tag="state")
            for n in range(NSUB):
                nc.sync.dma_start(out=st[:, n * NT:(n + 1) * NT],
                                  in_=stateT[:, n * NT:(n + 1) * NT])

            # Persistent PSUM accumulators, rotated manually. Banks are
            # zeroed by DVE several groups before reuse, and matmul groups
            # run WITHOUT start=True: the group-start bank-clear blocks the
            # LDWEIGHTS pull-ahead and costs ~100ns per group.
            NPS = 6
            ps_tiles = [psum_pool.tile([128, NT], F32, tag=f"ps{i}",
                                       name=f"ps{i}")
                        for i in range(NPS)]
            ps_idx = [0]

            def next_ps():
                t = ps_tiles[ps_idx[0] % NPS]
                ps_idx[0] += 1
                return t

            # ---- PE warmup: dummy matmuls on a zeroed tile so the HAM
            # clock-gate opens during the initial state/weight DMA window.
            # Also zeroes all accumulator banks for the no-start scheme.
            wz = consts.tile([128, NT], F16, tag="warmzero")
            nc.vector.memset(wz, 0.0)
            for t in ps_tiles:
                nc.vector.memset(t, 0.0)
            NWARM = 34
            wps = psum3_pool.tile([ACT_DIM, NT], F32, tag="p3_0",
                                  name="warm_ps")
            for w in range(NWARM):
                nc.tensor.matmul(
                    wps, wz[:, :ACT_DIM], wz,
                    start=(w == 0), stop=(w == NWARM - 1),
                    skip_group_check=True)

            w3sb = consts.tile([128, K3, ACT_DIM], F16, tag="w3")
            nc.sync.dma_start(out=w3sb, in_=w3t[:, :, :])

            # W1 resident in SBUF (one block per m-tile, loaded once)
            w1sb = []
            for m in range(M1):
                w1m = w1p.tile([128, K1 * 128], F16, tag=f"w1_{m}",
                               name=f"w1_{m}")
                nc.sync.dma_start(
                    out=w1m, in_=w1t[m].rearrange("p k j -> p (k j)"))
                w1sb.append(w1m)

            spine = [acts.tile([128, BL], F16, tag=f"spine{k}",
                               name=f"spine{k}")
                     for k in range(K1)]
            h1 = [acts.tile([128, BL], F16, tag=f"h1_{m}", name=f"h1_{m}")
                  for m in range(M1)]

            # layer-3 matmuls lag two (m, n) groups behind their relu so the
            # PE never waits on a freshly written h2 tile.
            pending = []

            def emit_l3(args):
                m, n, h2m, p3 = args
                nc.tensor.matmul(
                    p3, w3sb[:, m, :], h2m,
                    start=(m == 0), stop=(m == M2 - 1),
                    skip_group_check=True)
                if m == M2 - 1:
                    ot = outp.tile([ACT_DIM, NT], F32, tag="ot",
                                   name=f"ot_{n}")
                    nc.scalar.activation(
                        ot, p3, mybir.ActivationFunctionType.Tanh,
                        bias=sc[:ACT_DIM, 52:53])
                    nc.sync.dma_start(out=out[:, n * NT:(n + 1) * NT],
                                      in_=ot)

            # ---- fully interleaved per-column-chunk sweeps:
            # sigmoid(n) -> L1 m-sweep(n) -> L2 m-sweep(n) [+ lagged L3/tanh]
            for n in range(NSUB):
                ns = slice(n * NT, (n + 1) * NT)
                for k in range(K1):
                    nc.scalar.activation(
                        spine[k][:, ns], st[:, ns],
                        mybir.ActivationFunctionType.Sigmoid,
                        bias=sc[:, 10 + k:11 + k], scale=sc[:, k:k + 1])

                for m in range(M1):
                    ps = next_ps()
                    for k in range(K1):
                        nc.tensor.matmul(
                            ps, w1sb[m][:, k * 128:(k + 1) * 128],
                            spine[k][:, ns],
                            start=False, stop=False, skip_group_check=True)
                    nc.vector.tensor_scalar(
                        h1[m][:, ns], ps, sc[:, 20 + m:21 + m], 0.0, ADD, MAX)
                    nc.vector.memset(ps, 0.0)

                p3 = psum3_pool.tile([ACT_DIM, NT], F32, tag=f"p3_{n % 2}",
                                     name=f"p3_{n}")
                for m in range(M2):
                    w2sb = w2p.tile([128, K2 * 128], F16, tag="w2")
                    nc.sync.dma_start(
                        out=w2sb, in_=w2t[m].rearrange("p k j -> p (k j)"))
                    ps = next_ps()
                    for k in range(K2):
                        nc.tensor.matmul(
                            ps, w2sb[:, k * 128:(k + 1) * 128],
                            h1[k][:, ns],
                            start=False, stop=False, skip_group_check=True)
                    h2m = h2p.tile([128, NT], F16, tag="h2")
                    nc.vector.tensor_scalar(
                        h2m, ps, sc[:, 36 + m:37 + m], 0.0, ADD, MAX)
                    nc.vector.memset(ps, 0.0)
                    pending.append((m, n, h2m, p3))
                    if len(pending) > 2:
                        emit_l3(pending.pop(0))
            while pending:
                emit_l3(pending.pop(0))

    nc.compile()
    _cached["nc"] = nc
    return nc


def _prep_inputs(state, mean_enc, std_enc, W1, b1, W2, b2, W3, b3, Wd, bd):
    f32 = np.float32
    state = np.asarray(state, f32)
    mean_enc = np.asarray(mean_enc, f32)
    std_enc = np.asarray(std_enc, f32)
    W1 = np.asarray(W1, f32)
    b1 = np.asarray(b1, f32)
    W2 = np.asarray(W2, f32)
    b2 = np.asarray(b2, f32)
    W3 = np.asarray(W3, f32)
    b3 = np.asarray(b3, f32)
    Wd = np.asarray(Wd, f32)
    bd = np.asarray(bd, f32)

    # Fold decoder grouped conv into layer 3
    wd = Wd[:, 0, :]                                   # [32, 10]
    W3p = np.einsum("ak,akh->ah", wd, W3.reshape(ACT_DIM, DEC_K, H1))
    b3p = (b3.reshape(ACT_DIM, DEC_K) * wd).sum(1) + bd  # [32]

    # Permute encoder contraction: j' = k*128 + d
    W1p = W1.reshape(H0, D, ENC_K).transpose(0, 2, 1).reshape(H0, D * ENC_K)

    # Pre-tiled weight layouts: [m, p, k, j] = lhsT tile stack
    w1t = np.ascontiguousarray(
        W1p.reshape(M1, 128, K1, 128).transpose(0, 3, 2, 1).astype(np.float16))
    w2t = np.ascontiguousarray(
        W2.reshape(M2, 128, K2, 128).transpose(0, 3, 2, 1).astype(np.float16))
    w3t = np.ascontiguousarray(
        W3p.reshape(ACT_DIM, K3, 128).transpose(2, 1, 0).astype(np.float16))

    scal = np.zeros((128, 53), f32)
    scal[:, 0:10] = 1.0 / std_enc[0]                   # enc scale [128, 10]
    scal[:, 10:20] = -mean_enc[0] / std_enc[0]         # enc bias
    scal[:, 20:36] = b1.reshape(M1, 128).T
    scal[:, 36:52] = b2.reshape(M2, 128).T
    scal[:ACT_DIM, 52] = b3p

    in_maps = []
    for c in range(NCORES):
        shard = np.ascontiguousarray(state[c * BL:(c + 1) * BL].T)  # [128, BL]
        in_maps.append({
            "stateT": shard, "w1t": w1t, "w2t": w2t, "w3t": w3t, "scal": scal,
        })
    return in_maps


def kernel(**inputs):
    nc = _build_program()
    in_maps = _prep_inputs(**inputs)
    res = run_bass_kernel_spmd(nc, in_maps, core_ids=list(range(NCORES)))
    out = np.concatenate(
        [res.results[c]["out"].T for c in range(NCORES)], axis=0)
    return np.ascontiguousarray(out.astype(np.float32))


if __name__ == "__main__":
    rng = np.random.default_rng(0)
    state = rng.standard_normal((B, D), dtype=np.float32)
    mean = np.broadcast_to(
        np.linspace(-3, 3, ENC_K, dtype=np.float32), (1, D, ENC_K)).copy()
    std = np.full((1, D, ENC_K), 0.3872983346207417, np.float32)

    def lin(fan_in, fan_out):
        bound = 1 / np.sqrt(fan_in)
        return (rng.uniform(-bound, bound, (fan_out, fan_in)).astype(np.float32),
                rng.uniform(-bound, bound, fan_out).astype(np.float32))

    W1, b1 = lin(D * ENC_K, H0)
    W2, b2 = lin(H0, H1)
    W3, b3 = lin(H1, ACT_DIM * DEC_K)
    Wd = rng.uniform(-0.3, 0.3, (ACT_DIM, 1, DEC_K)).astype(np.float32)
    bd = rng.uniform(-0.3, 0.3, ACT_DIM).astype(np.float32)

    outp = kernel(state=state, mean_enc=mean, std_enc=std, W1=W1, b1=b1,
                  W2=W2, b2=b2, W3=W3, b3=b3, Wd=Wd, bd=bd)

    # numpy reference
    spine = 1 / (1 + np.exp(-(state[:, :, None] - mean) / std))
    a = np.maximum(spine.reshape(B, -1) @ W1.T + b1, 0)
    a = np.maximum(a @ W2.T + b2, 0)
    a = a @ W3.T + b3
    raw = np.einsum("bak,ak->ba", a.reshape(B, ACT_DIM, DEC_K), Wd[:, 0]) + bd
    ref = np.tanh(raw)
    rel = np.linalg.norm(outp - ref) / np.linalg.norm(ref)
    print("rel err:", rel, "max abs diff:", np.abs(outp - ref).max())
